# revision 2
# baseline (speedup 1.0000x reference)
"""Trainium2 Bass kernel for nn_DualFeatureExtractionStructureBlock (v2).

Self-contained. Strategy:
- Data-parallel over batch across 8 NeuronCores (4 batches/core); the
  neighbor axis folds into batch. Small weights replicated per core.
- Host prep: conv im2col, LN2-affine + fco folded into conv4 weights,
  everything cast to bf16 for matmuls + 2x/4x DVE elementwise modes.
- Activation-table discipline: one Gelu region (convs, fc, fusion for
  ALL batches), then one ln/exp region (LN1, attention, LN2, conv4)
  -> ~2 ACT table loads per rep instead of ~116. LN rstd computed as
  exp(-0.5*ln(E[x^2]+eps)) so Sqrt never loads a table.
- Softmax transposes done by the DMA xbar (SBUF->SBUF, 16x128 tiles)
  instead of PE+Pool; k-bias dropped (softmax-invariant); LN stats for
  sum and sum-of-squares accumulated into one [2,S2] psum via
  two-column one-hot lhs weights.
- PSUM: region A uses a scoped 4-deep matmul ring; BCD region uses
  {mm:2, scores:1, LN1 stats/bcast:2, LN2 stats/bcast:2, ao:1}.
- conv4 accumulation lives in SBUF (DVE add) to free a psum bank.
"""
import numpy as np
import ml_dtypes
from contextlib import ExitStack

import concourse.bass as bass
import concourse.mybir as mybir
from concourse.tile import TileContext
from concourse.vector_clock import ScopedClock
from concourse import tile as _tile_mod

F32 = mybir.dt.float32
F32R = mybir.dt.float32r
BF16 = mybir.dt.bfloat16
AF = mybir.ActivationFunctionType
ALU = mybir.AluOpType

S = 256
S2 = 2 * S
D = 128
N = 10
NPAIR = 5
INV_SQRT_DK = float(1.0 / (128.0 ** 0.5))
EPS1, EPS2 = 1e-6, 1e-5


def _build_kernel(b_loc=4, ln1_identity=True, split_waits=True, reps=1,
                  bufsA=4):
    nc = bass.Bass("TRN2")

    dt_in = {
        "tgt_im": ([b_loc, 12, S], BF16),
        "arr_im": ([b_loc, NPAIR, 12, S2], BF16),
        "W1": ([12, D], BF16), "W2": ([12, D], BF16),
        "b1": ([D, 1], F32), "b2": ([D, 1], F32),
        "fc1_w": ([D, D], BF16), "fc2_w": ([D, D], BF16),
        "fc1_b": ([D, 1], F32), "fc2_b": ([D, 1], F32),
        "fus_w": ([384, 384], BF16), "fus_b": ([D, 3], F32),
        "wq": ([384, 384], BF16), "wk": ([384, 384], BF16),
        "wv": ([384, 384], BF16), "wo": ([384, 384], BF16),
        "bq": ([D, 3], F32),
        "bv_bc": ([D, 384], F32), "bo": ([D, 3], F32),
        "ln1_g": ([D, 3], F32), "ln1_b": ([D, 3], F32),
        "W4f": ([N, 384, D], BF16), "b4f": ([D, 1], F32),
        "ident": ([D, D], F32R), "ident16": ([D, D], BF16),
        "ones_col16": ([D, 1], BF16),
        "ones_row16": ([1, D], BF16),
        "eps1": ([1, 1], F32), "eps2": ([1, 1], F32),
    }
    din = {k: nc.dram_tensor(k, shp, dt, kind="ExternalInput")
           for k, (shp, dt) in dt_in.items()}
    out_d = nc.dram_tensor("out", [b_loc, S, D], F32, kind="ExternalOutput")

    with TileContext(nc) as tc, ExitStack() as ctx:
        wpool = ctx.enter_context(tc.tile_pool(name="w", bufs=1))
        sbA = ctx.enter_context(tc.tile_pool(name="sbA", bufs=2))   # stream
        sbR = ctx.enter_context(tc.tile_pool(name="sbR", bufs=1))   # r per pair
        sbH = ctx.enter_context(tc.tile_pool(name="sbH", bufs=1))   # h per pair
        sbB = ctx.enter_context(tc.tile_pool(name="sbB", bufs=2))   # transients
        sbC = ctx.enter_context(tc.tile_pool(name="sbC", bufs=3))   # attn stream
        rows = ctx.enter_context(tc.tile_pool(name="rows", bufs=2))
        # one PSUM pool: a uniform 8-deep round-robin ring over all 8 banks.
        # Every psum tile is transient (written then drained within a few
        # allocations), so a global ring maximizes cross-pair pipelining.
        ps8 = ctx.enter_context(tc.tile_pool(name="ps8", bufs=1, space="PSUM"))
        _ps_idx = [0]

        def ps_next(shape, name, dt=F32):
            tag = f"pb{_ps_idx[0] % 8}"
            _ps_idx[0] += 1
            return ps8.tile(shape, dt, tag=tag, name=name, bufs=1)

        # ---- input DMAs first so compute can start immediately ----
        tgt_tiles, aim_tiles = {}, {}

        def load_inputs():
            for b in range(b_loc):
                t = sbA.tile([12, S], BF16, tag=f"tgt{b}", name=f"tgt{b}",
                             bufs=1)
                nc.sync.dma_start(out=t[:], in_=din["tgt_im"].ap()[b])
                tgt_tiles[b] = t
                for p in range(NPAIR):
                    a = sbA.tile([12, S2], BF16, tag=f"aim{b}{p}",
                                 name=f"aim{b}{p}", bufs=1)
                    nc.sync.dma_start(out=a[:], in_=din["arr_im"].ap()[b, p])
                    aim_tiles[(b, p)] = a

        load_inputs()

        def wtile(name, shape, dt=BF16, rearr=None):
            t = wpool.tile(shape, dt, name=name, tag=name)
            src = din[name].ap()
            if rearr is not None:
                src = src.rearrange(rearr[0], **rearr[1])
            nc.sync.dma_start(out=t[:], in_=src)
            return t

        # gelu-phase weights first; attention weights stream in behind
        W1 = wtile("W1", [12, D])
        W2 = wtile("W2", [12, D])
        b1 = wtile("b1", [D, 1], F32)
        b2 = wtile("b2", [D, 1], F32)
        fc1_w = wtile("fc1_w", [D, D])
        fc2_w = wtile("fc2_w", [D, D])
        fc1_b = wtile("fc1_b", [D, 1], F32)
        fc2_b = wtile("fc2_b", [D, 1], F32)
        fus_w = wtile("fus_w", [128, 3, 384], rearr=("(c p) o -> p c o", dict(p=128)))
        fus_b = wtile("fus_b", [D, 3], F32)
        ones_col16 = wtile("ones_col16", [D, 1])
        eps_t = {EPS1: wtile("eps1", [1, 1], F32),
                 EPS2: wtile("eps2", [1, 1], F32)}
        ones_row16 = wtile("ones_row16", [1, D])
        ident = wtile("ident", [D, D], F32R)
        ident16 = wtile("ident16", [D, D])
        ln1_g = wtile("ln1_g", [D, 3], F32)
        ln1_b = wtile("ln1_b", [D, 3], F32)
        bq = wtile("bq", [D, 3], F32)
        wq = wtile("wq", [128, 3, 384], rearr=("(c p) o -> p c o", dict(p=128)))
        wk = wtile("wk", [128, 3, 384], rearr=("(c p) o -> p c o", dict(p=128)))
        wv = wtile("wv", [128, 3, 384], rearr=("(c p) o -> p c o", dict(p=128)))
        bv_bc = wtile("bv_bc", [D, 384], F32)
        wo = wtile("wo", [128, 3, 384], rearr=("(c p) o -> p c o", dict(p=128)))
        bo = wtile("bo", [D, 3], F32)
        W4f = wtile("W4f", [128, N, 3, D], rearr=("n (c p) o -> p n c o", dict(p=128)))
        b4f = wtile("b4f", [D, 1], F32)

        # ---------------- region A: gelu table ----------------
        def prologue(b):
            ps = ps_next([128, S], "ps_c1")
            nc.tensor.matmul(ps[:], W1[:], tgt_tiles[b][:], start=True, stop=True)
            x1T = sbA.tile([128, S], BF16, tag="x1T", name="x1T")
            nc.scalar.activation(x1T[:], ps[:], AF.Gelu, bias=b1[:])
            ps = ps_next([128, S], "ps_f1")
            nc.tensor.matmul(ps[:], fc1_w[:], x1T[:], start=True, stop=True)
            t1 = sbA.tile([128, S], BF16, tag="t1", name="t1")
            nc.scalar.activation(t1[:], ps[:], AF.Gelu, bias=fc1_b[:])
            xm1 = sbA.tile([128, S], BF16, tag="xm1", name="xm1")
            nc.gpsimd.tensor_tensor(xm1[:], t1[:], x1T[:], op=ALU.mult)
            xmd = sbA.tile([128, S2], BF16, tag=f"xmd{b % 2}", name=f"xmd{b}",
                           bufs=1)
            nc.gpsimd.tensor_copy(xmd[:, 0:S], xm1[:])
            nc.gpsimd.tensor_copy(xmd[:, S:S2], xm1[:])
            return xmd

        def phaseA(b, p, xmd):
            ps = ps_next([128, S2], "ps_c2")
            nc.tensor.matmul(ps[:], W2[:], aim_tiles[(b, p)][:], start=True, stop=True)
            x2T = sbA.tile([128, S2], BF16, tag="x2T", name="x2T")
            nc.scalar.activation(x2T[:], ps[:], AF.Gelu, bias=b2[:])
            ps = ps_next([128, S2], "ps_f2")
            nc.tensor.matmul(ps[:], fc2_w[:], x2T[:], start=True, stop=True)
            t2 = sbA.tile([128, S2], BF16, tag="t2", name="t2")
            nc.scalar.activation(t2[:], ps[:], AF.Gelu, bias=fc2_b[:])
            y_mut = sbA.tile([128, S2], BF16, tag="ymut", name="y_mut")
            nc.gpsimd.tensor_tensor(y_mut[:], t2[:], x2T[:], op=ALU.mult)
            c2 = sbA.tile([128, S2], BF16, tag="c2", name="c2")
            nc.gpsimd.tensor_tensor(c2[:], xmd[:], y_mut[:], op=ALU.mult)
            cT = [xmd, y_mut, c2]
            r = []
            for mc in range(3):
                ps = ps_next([128, S2], "ps_g")
                for kc in range(3):
                    nc.tensor.matmul(ps[:], fus_w[:, kc, mc * 128:(mc + 1) * 128],
                                     cT[kc][:], start=(kc == 0), stop=(kc == 2))
                gel = sbB.tile([128, S2], BF16, tag=f"gel{mc}", name=f"gel{mc}")
                nc.scalar.activation(gel[:], ps[:], AF.Gelu,
                                     bias=fus_b[:, mc:mc + 1])
                rr_ = sbR.tile([128, S2], BF16, tag=f"r{b % 2}{p}{mc}",
                               name=f"r{b}{p}{mc}")
                nc.gpsimd.tensor_tensor(rr_[:], gel[:], cT[mc][:], op=ALU.add)
                r.append(rr_)
            return r

        # ---------------- region BCD: ln/exp table ----------------
        def layernorm(r, eps, out_tag, out_pool,
                      gamma=None, beta=None):
            """Partition-axis LN over 3 bf16 chunks r[mc] [128, S2].

            Stats: one [2,S2] psum (row0 = sum, row1 = sum of squares) via
            one-hot two-column lhs weights. rstd = exp(-0.5*ln(ss/384+eps)).
            xhat = r*R + P with R/P broadcast by rank-1 matmuls, staged to
            bf16 SBUF so the DVE applies run in 2x mode.
            """
            sq = [sbB.tile([128, S2], BF16, tag=f"lnsq{mc}", name=f"lnsq{mc}")
                  for mc in range(3)]
            for mc in range(3):
                nc.gpsimd.tensor_tensor(sq[mc][:], r[mc][:], r[mc][:], op=ALU.mult)
            ps_sum = ps_next([1, S2], "ps_sum")
            for mc in range(3):
                nc.tensor.matmul(ps_sum[:], ones_col16[:], r[mc][:],
                                 start=(mc == 0), stop=(mc == 2))
            ps_ss = ps_next([1, S2], "ps_ss")
            for mc in range(3):
                nc.tensor.matmul(ps_ss[:], ones_col16[:], sq[mc][:],
                                 start=(mc == 0), stop=(mc == 2))
            ln_row = rows.tile([1, S2], F32, tag="lnrow", name="ln_row")
            nc.scalar.activation(ln_row[:], ps_ss[:], AF.Ln,
                                 scale=1.0 / 384.0, bias=eps_t[eps][:])
            rstd_row = rows.tile([1, S2], BF16, tag="rstdrow", name="rstd_row")
            nc.scalar.activation(rstd_row[:], ln_row[:], AF.Exp, scale=-0.5)
            p_row = rows.tile([1, S2], BF16, tag="prow", name="p_row")
            nc.vector.scalar_tensor_tensor(p_row[:], ps_sum[:], -1.0 / 384.0,
                                           rstd_row[:], op0=ALU.mult, op1=ALU.mult)
            Rb = ps_next([128, S2], "Rb")
            nc.tensor.matmul(Rb[:], ones_row16[:], rstd_row[:], start=True, stop=True)
            Rs = sbB.tile([128, S2], BF16, tag="lnRs", name="Rs")
            nc.scalar.activation(Rs[:], Rb[:], AF.Identity)
            Pb = ps_next([128, S2], "Pb")
            nc.tensor.matmul(Pb[:], ones_row16[:], p_row[:], start=True, stop=True)
            Ps = sbB.tile([128, S2], BF16, tag="lnPs", name="Ps")
            nc.scalar.activation(Ps[:], Pb[:], AF.Identity)
            h = [out_pool.tile([128, S2], BF16, tag=f"{out_tag}{mc}",
                               name=f"{out_tag}{mc}") for mc in range(3)]
            u = [sbB.tile([128, S2], BF16, tag=f"lnu{mc}", name=f"lnu{mc}")
                 for mc in range(3)]
            for mc in range(3):
                nc.vector.tensor_tensor(u[mc][:], r[mc][:], Rs[:], op=ALU.mult)
                nc.vector.tensor_tensor(h[mc][:], u[mc][:], Ps[:], op=ALU.add)
                if gamma is not None:
                    nc.scalar.activation(h[mc][:], h[mc][:], AF.Identity,
                                         bias=beta[:, mc:mc + 1],
                                         scale=gamma[:, mc:mc + 1])
            return h

        def phaseB(b, p, r):
            return layernorm(r, EPS1, f"h{p}", sbH,
                             gamma=None if ln1_identity else ln1_g,
                             beta=None if ln1_identity else ln1_b)

        def phaseC_qkv(b, p, h):
            qT, kT = [], []
            for mc in range(3):
                ps = ps_next([128, S2], "ps_q")
                for kc in range(3):
                    nc.tensor.matmul(ps[:], wq[:, kc, mc * 128:(mc + 1) * 128],
                                     h[kc][:], start=(kc == 0), stop=(kc == 2))
                q = sbC.tile([128, S2], BF16, tag=f"qT{mc}", name=f"qT{mc}",
                             bufs=2)
                nc.scalar.activation(q[:], ps[:], AF.Identity, bias=bq[:, mc:mc + 1])
                qT.append(q)
            for mc in range(3):
                ps = ps_next([128, S2], "ps_k")
                for kc in range(3):
                    nc.tensor.matmul(ps[:], wk[:, kc, mc * 128:(mc + 1) * 128],
                                     h[kc][:], start=(kc == 0), stop=(kc == 2))
                k = sbC.tile([128, S2], BF16, tag=f"kT{mc}", name=f"kT{mc}",
                             bufs=2)
                nc.vector.tensor_copy(k[:], ps[:])
                kT.append(k)
            v = []
            for sig in range(2):
                vs = []
                for sc in range(2):
                    psv = ps_next([128, 384], "psv")
                    off = sig * S + sc * 128
                    for kc in range(3):
                        nc.tensor.matmul(psv[:], h[kc][:, off:off + 128],
                                         wv[:, kc, :],
                                         start=(kc == 0), stop=(kc == 2))
                    vt = sbC.tile([128, 384], BF16, tag=f"v{sig}{sc}",
                                  name=f"v{sig}{sc}", bufs=2)
                    nc.vector.tensor_tensor(vt[:], psv[:], bv_bc[:], op=ALU.add)
                    vs.append(vt)
                v.append(vs)
            return qT, kT, v

        def phaseC_attn(b, p, qkv):
            qT, kT, v = qkv
            aoT = []
            for hd in range(3):
                ao_ps = ps_next([128, S2], "ao_ps")
                for sig in range(2):
                    ps_s = ps_next([128, S2], "ps_s")
                    rs = sbC.tile([128, 2], F32, tag="rs", name="rs")
                    E = sbC.tile([128, S2], BF16, tag="E", name="E")
                    for qc in range(2):
                        nc.tensor.matmul(
                            ps_s[:, qc * S:(qc + 1) * S],
                            qT[hd][:, sig * S + qc * 128: sig * S + (qc + 1) * 128],
                            kT[hd][:, sig * S:(sig + 1) * S],
                            start=True, stop=True)
                        nc.scalar.activation(E[:, qc * S:(qc + 1) * S],
                                             ps_s[:, qc * S:(qc + 1) * S],
                                             AF.Exp, scale=INV_SQRT_DK,
                                             accum_out=rs[:, qc:qc + 1])
                    rrec = sbC.tile([128, 2], F32, tag="rr", name="rrec")
                    nc.vector.reciprocal(rrec[:], rs[:])
                    A = sbC.tile([128, S2], BF16, tag="A", name="A")
                    for qc in range(2):
                        nc.vector.tensor_scalar_mul(A[:, qc * S:(qc + 1) * S],
                                                    E[:, qc * S:(qc + 1) * S],
                                                    rrec[:, qc:qc + 1])
                    ps_t = ps_next([128, S2], "ps_t", dt=BF16)
                    for sc in range(2):
                        for qc in range(2):
                            nc.tensor.transpose(
                                ps_t[:, sc * S + qc * 128: sc * S + (qc + 1) * 128],
                                A[:, qc * S + sc * 128: qc * S + (sc + 1) * 128],
                                ident16[:])
                    At2 = sbC.tile([128, S2], BF16, tag="At2", name="At2")
                    nc.vector.tensor_copy(At2[:], ps_t[:])
                    for sc in range(2):
                        nc.tensor.matmul(
                            ao_ps[:, sig * S:(sig + 1) * S],
                            v[sig][sc][:, hd * 128:(hd + 1) * 128],
                            At2[:, sc * S:(sc + 1) * S],
                            start=(sc == 0), stop=(sc == 1))
                ao_t = sbB.tile([128, S2], BF16, tag=f"ao{hd}", name=f"ao{hd}")
                nc.vector.tensor_copy(ao_t[:], ao_ps[:])
                aoT.append(ao_t)
            return aoT

        def phaseD(b, p, h, aoT, acc):
            r2 = []
            for mc in range(3):
                ps_o = ps_next([128, S2], "ps_o")
                for kc in range(3):
                    nc.tensor.matmul(ps_o[:], wo[:, kc, mc * 128:(mc + 1) * 128],
                                     aoT[kc][:], start=(kc == 0), stop=(kc == 2))
                r2t = sbB.tile([128, S2], BF16, tag=f"r2{mc}", name=f"r2{mc}")
                nc.vector.scalar_tensor_tensor(
                    r2t[:], ps_o[:], bo[:, mc:mc + 1], h[mc][:],
                    op0=ALU.add, op1=ALU.add)
                r2.append(r2t)
            h2 = layernorm(r2, EPS2, "h2", sbB)
            ps_w = ps_next([128, S], "ps_w4")
            for sig in range(2):
                for kc in range(3):
                    nc.tensor.matmul(ps_w[:], W4f[:, 2 * p + sig, kc, :],
                                     h2[kc][:, sig * S:(sig + 1) * S],
                                     start=(sig == 0 and kc == 0),
                                     stop=(sig == 1 and kc == 2))
            if p == 0:
                nc.vector.tensor_copy(acc[:], ps_w[:])
            else:
                nc.vector.tensor_tensor(acc[:], acc[:], ps_w[:], op=ALU.add)

        def epilogue(b, acc):
            outT = sbA.tile([128, S], F32R, tag="outT", name="outT")
            nc.scalar.activation(outT[:], acc[:], AF.Identity, bias=b4f[:])
            for sc in range(2):
                ps_t = ps_next([128, 128], "ps_ot", dt=F32R)
                nc.tensor.transpose(ps_t[:], outT[:, sc * 128:(sc + 1) * 128],
                                    ident[:])
                o_sb = sbA.tile([128, 128], F32, tag=f"oseq{sc}", name=f"oseq{sc}")
                nc.vector.tensor_copy(o_sb[:], ps_t[:].bitcast(F32))
                nc.sync.dma_start(out=out_d.ap()[b, sc * 128:(sc + 1) * 128, :],
                                  in_=o_sb[:])

        GRP = 2   # batches per region-alternation group
        for _rep in range(reps):
            if _rep > 0:
                load_inputs()
            for g0 in range(0, b_loc, GRP):
                batches = range(g0, min(g0 + GRP, b_loc))
                rs_all = {}
                for b in batches:
                    xmd = prologue(b)
                    for p in range(NPAIR):
                        rs_all[(b, p)] = phaseA(b, p, xmd)
                pairs = [(b, p) for b in batches for p in range(NPAIR)]
                accs = {}
                prev = None
                for (b, p) in pairs:
                    if p == 0:
                        accs[b] = sbA.tile([128, S], F32, tag="accsb",
                                           name=f"accsb{b}", bufs=2)
                    h = phaseB(b, p, rs_all.pop((b, p)))
                    qkv = phaseC_qkv(b, p, h)
                    if prev is not None:
                        pb, pp, ph, pqkv = prev
                        aoT = phaseC_attn(pb, pp, pqkv)
                        phaseD(pb, pp, ph, aoT, accs[pb])
                        if pp == NPAIR - 1:
                            epilogue(pb, accs[pb])
                    prev = (b, p, h, qkv)
                pb, pp, ph, pqkv = prev
                aoT = phaseC_attn(pb, pp, pqkv)
                phaseD(pb, pp, ph, aoT, accs[pb])
                epilogue(pb, accs[pb])

    if split_waits:
        split_multiwaits(nc)
    return nc


# ---------------- walrus compat patches ----------------


def _patched_drain_and_barrier(self, tick_clock, wait_clock):
    nc = self.nc
    probe = nc.sync.nop(nofuse=True)
    wait_clock.add_sem_waits(probe.ins, ScopedClock({None: tick_clock.global_clock}))
    si = probe.ins.sync_info
    waits = list(si.on_wait) if si is not None else []
    if len(waits) > 1:
        probe.ins.sync_info = mybir.SyncInfo(on_wait=[waits[0]], on_update=[])
        for w in waits[1:]:
            n = nc.sync.nop(nofuse=True)
            n.ins.sync_info = mybir.SyncInfo(on_wait=[w], on_update=[])
    nc.sync.drain()
    nc.all_engine_barrier()
    assert self.sems is not None
    popped = nc._tile_sem_poison_stack.pop()
    assert popped is self._sem_poison
    nc.clear_and_free_semaphores(list(self.sems.allocated().values()))
    nc.all_engine_barrier()


_tile_mod.TileContext._drain_and_barrier = _patched_drain_and_barrier


def split_multiwaits(nc):
    n_split = 0
    for fn in nc.m.functions:
        for bb in fn.blocks:
            needs = False
            for ins in bb.instructions:
                si = ins.sync_info
                if si is not None and len(si.on_wait) > 1:
                    needs = True
                    break
            if not needs:
                continue
            new_list = []
            for ins in bb.instructions:
                si = ins.sync_info
                if si is not None and len(si.on_wait) > 1:
                    waits = list(si.on_wait)
                    for w in waits[:-1]:
                        nop = mybir.InstNoOp(
                            name=f"waitsplit-{n_split}",
                            ins=[],
                            outs=[],
                        )
                        nop.engine = ins.engine
                        nop.sync_info = mybir.SyncInfo(on_wait=[w], on_update=[])
                        new_list.append(nop)
                        n_split += 1
                    ins.sync_info = mybir.SyncInfo(
                        on_wait=[waits[-1]], on_update=list(si.on_update)
                    )
                new_list.append(ins)
            bb.instructions = new_list
    return n_split


# ---------------- host prep ----------------

B, S, N, D = 32, 256, 10, 128
D3 = 3 * D
NCORES = 8
B_LOC = B // NCORES          # 4 batches per core
NPAIR = N // 2               # 5 neighbor pairs per batch
NPF32 = np.float32
BF = ml_dtypes.bfloat16


def _im2col(ch):
    """ch: [..., 4, S] channel-major sequences -> [..., 12, S] rows f=c*3+t."""
    lead = ch.shape[:-2]
    out = np.zeros(lead + (12, S), NPF32)
    for c in range(4):
        for t in range(3):
            lo, hi = max(0, 1 - t), min(S, S + 1 - t)
            out[..., c * 3 + t, lo:hi] = ch[..., c, lo + t - 1:hi + t - 1]
    return out


def prep_host(inputs):
    """Returns dict of device arrays (full batch) + metadata."""
    x = np.asarray(inputs["x"], NPF32)                      # [B, S, 44]
    tgt = np.ascontiguousarray(x[..., :4].transpose(0, 2, 1))   # [B, 4, S]
    arr = np.ascontiguousarray(
        x[..., 4:].transpose(0, 2, 1).reshape(B, N, 4, S))      # [B, N, 4, S]

    tgt_im = _im2col(tgt)                                  # [B, 12, S]
    arr_im_seq = _im2col(arr)                              # [B, N, 12, S]
    arr_im = np.ascontiguousarray(
        arr_im_seq.reshape(B, NPAIR, 2, 12, S).transpose(0, 1, 3, 2, 4)
    ).reshape(B, NPAIR, 12, 2 * S)

    g = lambda k: np.asarray(inputs[k], NPF32)

    W1 = np.ascontiguousarray(g("conv1_w").transpose(1, 2, 0).reshape(12, D))
    W2 = np.ascontiguousarray(g("conv2_w").transpose(1, 2, 0).reshape(12, D))

    ln2_g, ln2_b = g("ln2_g"), g("ln2_b")
    fco_w2 = ln2_g[:, None] * g("fco_w")                   # [384, 128]
    fco_b2 = ln2_b @ g("fco_w") + g("fco_b")               # [128]

    W4c = g("conv4_w")[:, :, 0, :]                         # [o, c, n]
    W4n = np.stack([W4c[:, :, n].T for n in range(N)])      # [N, c, o]
    W4f = np.stack([fco_w2 @ W4n[n] for n in range(N)])     # [N, 384, o]
    b4f = g("conv4_b") + sum(W4n[n].T @ fco_b2 for n in range(N))  # [o]

    ln1_g, ln1_b = g("ln1_g"), g("ln1_b")
    ln1_identity = bool(np.all(ln1_g == 1.0) and np.all(ln1_b == 0.0))

    def chunked(v):  # [384] -> [128, 3] per-partition chunks
        return np.ascontiguousarray(v.reshape(3, 128).T)

    bf = lambda a: np.ascontiguousarray(a).astype(BF)

    dev = {
        "tgt_im": bf(tgt_im),
        "arr_im": bf(arr_im),
        "W1": bf(W1), "W2": bf(W2),
        "b1": g("conv1_b").reshape(D, 1), "b2": g("conv2_b").reshape(D, 1),
        "fc1_w": bf(g("fc1_w")), "fc2_w": bf(g("fc2_w")),
        "fc1_b": g("fc1_b").reshape(D, 1), "fc2_b": g("fc2_b").reshape(D, 1),
        "fus_w": bf(g("fus_w")), "fus_b": chunked(g("fus_b")),
        "wq": bf(g("wq")), "wk": bf(g("wk")), "wv": bf(g("wv")),
        "wo": bf(g("wo")),
        "bq": chunked(g("bq")),
        "bv_bc": np.ascontiguousarray(
            np.broadcast_to(g("bv")[None, :], (D, D3))).astype(NPF32),
        "bo": chunked(g("bo")),
        "ln1_g": chunked(ln1_g), "ln1_b": chunked(ln1_b),
        "W4f": bf(W4f), "b4f": b4f.reshape(D, 1),
        "ident": np.eye(128, dtype=NPF32),
        "ident16": np.eye(128, dtype=BF),
        "ones_col16": np.ones((128, 1), BF),
        "ones_row16": np.ones((1, 128), BF),
        "eps1": np.full((1, 1), 1e-6, NPF32),
        "eps2": np.full((1, 1), 1e-5, NPF32),
    }
    return dev, ln1_identity


def shard(dev, core):
    """Per-core input map: batch-shard the activations, replicate weights."""
    s = slice(core * B_LOC, (core + 1) * B_LOC)
    m = dict(dev)
    m["tgt_im"] = np.ascontiguousarray(dev["tgt_im"][s])
    m["arr_im"] = np.ascontiguousarray(dev["arr_im"][s])
    return m


# ---------------- runner ----------------
import jax
from jax.sharding import Mesh, PartitionSpec
try:
    from jax.experimental.shard_map import shard_map
except Exception:
    from jax.shard_map import shard_map

from concourse import bass2jax
from concourse.bass2jax import _bass_exec_p, install_neuronx_cc_hook, partition_id_tensor


def make_runner(nc, n_cores=8):
    install_neuronx_cc_hook()
    partition_name = nc.partition_id_tensor.name if nc.partition_id_tensor else None

    in_names, out_names, out_avals, zero_outs = [], [], [], []
    for alloc in nc.m.functions[0].allocations:
        if not isinstance(alloc, mybir.MemoryLocationSet):
            continue
        name = alloc.memorylocations[0].name
        if alloc.kind == "ExternalInput":
            if name != partition_name:
                in_names.append(name)
        elif alloc.kind == "ExternalOutput":
            out_names.append(name)
            shape = tuple(alloc.tensor_shape)
            dtype = mybir.dt.np(alloc.dtype)
            out_avals.append(jax.core.ShapedArray(shape, dtype))
            zero_outs.append(np.zeros(shape, dtype))
    n_params = len(in_names)
    all_in_names = list(in_names) + list(out_names)
    if partition_name is not None:
        all_in_names.append(partition_name)

    def _body(*args):
        operands = list(args)
        if partition_name is not None:
            operands.append(partition_id_tensor())
        outs = _bass_exec_p.bind(
            *operands,
            out_avals=tuple(out_avals),
            in_names=tuple(all_in_names),
            out_names=tuple(out_names),
            lowering_input_output_aliases=(),
            sim_require_finite=True,
            sim_require_nnan=True,
            nc=nc,
        )
        return tuple(outs)

    devices = jax.devices()[:n_cores]
    mesh = Mesh(np.asarray(devices), ("core",))
    in_specs = (PartitionSpec("core"),) * (n_params + len(out_names))
    out_specs = (PartitionSpec("core"),) * len(out_names)
    fn = jax.jit(shard_map(_body, mesh=mesh, in_specs=in_specs,
                           out_specs=out_specs, check_rep=False),
                 keep_unused=True)

    def prepare(in_maps):
        per_core = [[np.asarray(m[name]) for name in in_names] for m in in_maps]
        concat_in = [np.concatenate([per_core[c][i] for c in range(n_cores)], axis=0)
                     for i in range(n_params)]
        concat_zeros = [np.zeros((n_cores * z.shape[0], *z.shape[1:]), z.dtype)
                        for z in zero_outs]
        args = [jax.device_put(a) for a in concat_in + concat_zeros]
        for a in args:
            a.block_until_ready()
        return args

    def run(args):
        outs = fn(*args)
        jax.block_until_ready(outs)
        return outs

    def gather(outs):
        return [
            {name: np.asarray(outs[i]).reshape(n_cores, *out_avals[i].shape)[c]
             for i, name in enumerate(out_names)}
            for c in range(n_cores)
        ]

    return prepare, run, gather


# ---------------- public entry ----------------
_CACHE = {}


def kernel(**inputs) -> np.ndarray:
    dev, ln1_id = prep_host(inputs)
    key = ("k", ln1_id)
    if key not in _CACHE:
        nc = _build_kernel(b_loc=B_LOC, ln1_identity=ln1_id)
        _CACHE[key] = make_runner(nc)
    prepare, run, gather = _CACHE[key]
    in_maps = [shard(dev, c) for c in range(NCORES)]
    args = prepare(in_maps)
    outs = run(args)
    res = gather(outs)
    out = np.concatenate([res[c]["out"] for c in range(NCORES)], axis=0)
    return out.astype(np.float32)
